# revision 1
# baseline (speedup 1.0000x reference)
"""Trainium2 Bass kernel for nn_EvolutionaryFeatureExtractor.

Reference computes, from a one-hot MSA (K=512, L=256, A=21):
  pssm         (L, A)  = log2(((mean + 0.001)/rowsum) * A)
  conservation (L,)    = 1 - entropy/log2(A)
  coevolution  (L, L)  = APC-corrected mutual information

Algebraic restructure used here (validated against the jax reference):
  joint[i,j,a,b] = C[i,j,a,b]/K + EPS with C the integer pair counts.
  sum_b joint[i,j,a,b] = p_raw[i,a] + A*EPS  (independent of j), so the
  two einsum('ijab,ia->ij') terms collapse to rank-1: MI = S - t_i - t_j
  with S[i,j] = sum_ab J*log2(J) and t[i] = sum_a (p_raw+A*EPS)*log2(p_raw+EPS).

Device pipeline per core (core c owns i-positions [32c, 32c+32)):
  counts matmul (fp8, exact for 0/1 one-hots): C^T[jb, ia] = M^T M block
  ACT:  G = ln(C/512 + 1e-9)
  DVE:  F = (G + SHIFT) * C   (fp16, recentred so fp16 error is tiny;
        the eps*G term is dropped -- ~1e-5 relative, validated)
  PE:   indicator matmul contracts jb partitions -> R[j, ia] = sum_b F
  DVE:  grouped reduce over a -> S_raw^T[j, i]
  plus tiny marginal path for pssm/conservation/t.
Host: gather, S = (S_raw - 512*SHIFT)/(512*ln2), MI/APC assembly (numpy,
~0.5 MFLOP of vector math on gathered results).

The j axis is padded A=21 -> 24 letters so jb rows tile 128 exactly
(pad rows have C=0 and F=0 so they are self-masking).
"""

import numpy as np
import ml_dtypes

import concourse.bass as bass
import concourse.mybir as mybir
import concourse.tile as tile
from concourse import bacc, bass_utils

F32 = mybir.dt.float32
F16 = mybir.dt.float16
FP8 = mybir.dt.float8e4  # e4m3: 0.0/1.0 exact
NP_FP8 = ml_dtypes.float8_e4m3

K, L, A = 512, 256, 21
AP24 = 24                  # padded alphabet for the jb (partition) axis
N_CORES = 8
IB = L // N_CORES          # 32 i-positions per core
NW = IB * A                # 672 rhs columns per core
MP = L * AP24              # 6144 lhsT columns (jb, padded)
NPT = MP // 128            # 48 jb partition tiles
KT = K // 128              # 4 contraction tiles
EPS = 1e-9
SHIFT = 6.0                # F = (ln J + SHIFT) * C recentring
LN2 = float(np.log(2.0))

_CACHE = {}


def _build():
    nc = bacc.Bacc("TRN2", target_bir_lowering=False, debug=False,
                   num_devices=N_CORES)

    lhst_d = nc.dram_tensor("lhst", [K, MP], FP8, kind="ExternalInput").ap()
    rhs_d = nc.dram_tensor("rhs", [K, NW], FP8, kind="ExternalInput").ap()
    ind_d = nc.dram_tensor("ind", [24, 128, 128], F16, kind="ExternalInput").ap()
    sout_d = nc.dram_tensor("sout", [L, IB], F32, kind="ExternalOutput").ap()
    pssm_d = nc.dram_tensor("pssm_raw", [1, NW], F32, kind="ExternalOutput").ap()
    traw_d = nc.dram_tensor("traw", [1, IB], F32, kind="ExternalOutput").ap()
    eraw_d = nc.dram_tensor("eraw", [1, IB], F32, kind="ExternalOutput").ap()

    with tile.TileContext(nc) as tc:
        with (
            tc.tile_pool(name="inp", bufs=1) as inp,
            tc.tile_pool(name="cpool", bufs=2, space="PSUM") as cpool,
            tc.tile_pool(name="rpool", bufs=1, space="PSUM") as rpool,
            tc.tile_pool(name="gpool", bufs=2) as gpool,
            tc.tile_pool(name="fpool", bufs=2) as fpool,
            tc.tile_pool(name="opool", bufs=1) as opool,
        ):
            kbuf = inp.tile([128, KT, MP], FP8)
            rbuf = inp.tile([128, KT, NW], FP8)
            ibuf = inp.tile([128, 24, 128], F16)
            for k in range(KT):
                nc.sync.dma_start(kbuf[:, k, :], lhst_d[k * 128:(k + 1) * 128, :])
                nc.sync.dma_start(rbuf[:, k, :], rhs_d[k * 128:(k + 1) * 128, :])
            for r in range(24):
                nc.sync.dma_start(ibuf[:, r, :], ind_d[r, :, :])

            bias1 = inp.tile([128, 1], F32)
            nc.vector.memset(bias1[:], EPS)
            biasp = inp.tile([128, 1], F32)
            nc.vector.memset(biasp[:], 0.001 * A / (1.0 + A * 0.001))
            ones8 = inp.tile([128, 1], FP8)
            nc.vector.memset(ones8[:], 1.0)

            r1a = rpool.tile([128, 1024], F32, tag="r1a")
            r1b = rpool.tile([128, 1024], F32, tag="r1b")

            for r in range(NPT):
                ctile = cpool.tile([128, 1024], F32, tag="c", name=f"c{r}")
                for k in range(KT):
                    lw = kbuf[:, k, r * 128:(r + 1) * 128]
                    nc.tensor.matmul(ctile[:, 0:512], lw, rbuf[:, k, 0:512],
                                     start=(k == 0), stop=(k == KT - 1))
                    nc.tensor.matmul(ctile[:, 512:NW], lw, rbuf[:, k, 512:NW],
                                     start=(k == 0), stop=(k == KT - 1))
                g = gpool.tile([128, NW], F32, tag="g", name=f"g{r}")
                nc.scalar.activation(g[:], ctile[:, 0:NW],
                                     mybir.ActivationFunctionType.Ln,
                                     scale=1.0 / K, bias=bias1[:])
                f = fpool.tile([128, NW], F16, tag="f", name=f"f{r}")
                nc.vector.scalar_tensor_tensor(
                    f[:], g[:], SHIFT, ctile[:, 0:NW],
                    op0=mybir.AluOpType.add, op1=mybir.AluOpType.mult)
                rt = r1a if r < NPT // 2 else r1b
                ind_ap = ibuf[:, r % 24, :]
                nc.tensor.matmul(rt[:, 0:512], ind_ap, f[:, 0:512],
                                 start=(r % 24 == 0), stop=(r % 24 == 23))
                nc.tensor.matmul(rt[:, 512:NW], ind_ap, f[:, 512:NW],
                                 start=(r % 24 == 0), stop=(r % 24 == 23))

            sa = opool.tile([128, IB], F32)
            nc.vector.reduce_sum(sa[:], r1a[:, 0:NW].rearrange("p (i a) -> p i a", a=A),
                                 axis=mybir.AxisListType.X)
            sb = opool.tile([128, IB], F32)
            nc.vector.reduce_sum(sb[:], r1b[:, 0:NW].rearrange("p (i a) -> p i a", a=A),
                                 axis=mybir.AxisListType.X)
            nc.sync.dma_start(sout_d[0:128, :], sa[:])
            nc.sync.dma_start(sout_d[128:L, :], sb[:])

            # marginal path: counts of the core's own columns
            cm = cpool.tile([1, 1024], F32, tag="c", name="cm")
            for k in range(KT):
                nc.tensor.matmul(cm[:, 0:512], ones8[:, 0:1], rbuf[:, k, 0:512],
                                 start=(k == 0), stop=(k == KT - 1))
                nc.tensor.matmul(cm[:, 512:NW], ones8[:, 0:1], rbuf[:, k, 512:NW],
                                 start=(k == 0), stop=(k == KT - 1))
            lnfe = gpool.tile([1, NW], F32, tag="g", name="lnfe")
            nc.scalar.activation(lnfe[:], cm[:, 0:NW],
                                 mybir.ActivationFunctionType.Ln,
                                 scale=1.0 / K, bias=bias1[0:1])
            pssm_t = fpool.tile([1, NW], F32, tag="f", name="pssm_t")
            nc.scalar.activation(pssm_t[:], cm[:, 0:NW],
                                 mybir.ActivationFunctionType.Ln,
                                 scale=A / (K * (1.0 + A * 0.001)),
                                 bias=biasp[0:1])
            nc.sync.dma_start(pssm_d[:, :], pssm_t[:])
            tv = fpool.tile([1, NW], F32, tag="f", name="tv")
            nc.vector.scalar_tensor_tensor(
                tv[:], cm[:, 0:NW], float(K * A * EPS), lnfe[:],
                op0=mybir.AluOpType.add, op1=mybir.AluOpType.mult)
            ev = gpool.tile([1, NW], F32, tag="g", name="ev")
            nc.vector.scalar_tensor_tensor(
                ev[:], cm[:, 0:NW], float(K * EPS), lnfe[:],
                op0=mybir.AluOpType.add, op1=mybir.AluOpType.mult)
            tr = opool.tile([1, IB], F32)
            nc.vector.reduce_sum(tr[:], tv[:].rearrange("p (i a) -> p i a", a=A),
                                 axis=mybir.AxisListType.X)
            er = opool.tile([1, IB], F32)
            nc.vector.reduce_sum(er[:], ev[:].rearrange("p (i a) -> p i a", a=A),
                                 axis=mybir.AxisListType.X)
            nc.sync.dma_start(traw_d[:, :], tr[:])
            nc.sync.dma_start(eraw_d[:, :], er[:])

    nc.compile()
    return nc


def _indicator():
    ind = np.zeros((24, 128, 128), np.float16)
    r = np.arange(24)[:, None]
    p = np.arange(128)[None, :]
    j = (128 * r + p) // AP24
    ind[r, p, j] = 1.0
    return ind


def run(msa: np.ndarray, trace: bool = False):
    """Shard, run the SPMD kernel on 8 cores, return (outputs, BassKernelResults)."""
    if "nc" not in _CACHE:
        _CACHE["nc"] = _build()
        _CACHE["ind"] = _indicator()
    nc = _CACHE["nc"]

    m21 = np.ascontiguousarray(msa.reshape(K, L * A)).astype(NP_FP8)
    m24 = np.zeros((K, MP), NP_FP8)
    m24.reshape(K, L, AP24)[:, :, :A] = msa
    ind = _CACHE["ind"]

    in_maps = [
        {"lhst": m24, "rhs": np.ascontiguousarray(m21[:, c * NW:(c + 1) * NW]),
         "ind": ind}
        for c in range(N_CORES)
    ]
    res = bass_utils.run_bass_kernel_spmd(
        nc, in_maps, core_ids=list(range(N_CORES)), trace=trace)
    return res


def assemble(res) -> tuple[np.ndarray, np.ndarray, np.ndarray]:
    inv = 1.0 / (K * LN2)
    # S^T blocks: sout[c][j, i_local] -> S[i, j]
    s = np.concatenate([res.results[c]["sout"].T for c in range(N_CORES)], axis=0)
    s = (s - np.float32(K * SHIFT)) * np.float32(inv)
    t = np.concatenate([res.results[c]["traw"][0] for c in range(N_CORES)]) * np.float32(inv)
    e = np.concatenate([res.results[c]["eraw"][0] for c in range(N_CORES)])
    pssm = np.concatenate([res.results[c]["pssm_raw"][0] for c in range(N_CORES)])

    mi = s - t[:, None] - t[None, :]
    np.fill_diagonal(mi, 0.0)
    rm = mi.mean(axis=1)
    cm = mi.mean(axis=0)
    tm = mi.mean()
    mi = mi - np.outer(rm, cm) / (tm + EPS)

    pssm = (pssm / np.float32(LN2)).reshape(L, A).astype(np.float32)
    conservation = (1.0 + e * inv / np.log2(A)).astype(np.float32)
    return pssm, conservation, mi.astype(np.float32)


def kernel(msa: np.ndarray):
    res = run(np.asarray(msa, dtype=np.float32), trace=False)
    return assemble(res)


# revision 2
# speedup vs baseline: 1.5388x; 1.5388x over previous
"""Trainium2 Bass kernel for nn_EvolutionaryFeatureExtractor.

Reference computes, from a one-hot MSA (K=512, L=256, A=21):
  pssm         (L, A)  = log2(((mean + 0.001)/rowsum) * A)
  conservation (L,)    = 1 - entropy/log2(A)
  coevolution  (L, L)  = APC-corrected mutual information

Algebraic restructure (validated against the jax reference):
  joint[i,j,a,b] = C[i,j,a,b]/K + EPS with C the integer pair counts.
  sum_b joint[i,j,a,b] = p_raw[i,a] + A*EPS  (independent of j), so the
  two einsum('ijab,ia->ij') terms collapse to rank-1: MI = S - t_i - t_j
  with S[i,j] = sum_ab J*log2(J) and t[i] = sum_a (p_raw+A*EPS)*log2(p_raw+EPS).
  S is symmetric, so each core only computes a wrap-around band of j.

Device pipeline per core (core c owns i-positions [32c, 32c+32) and the
j-window [32c, 32c+160) mod 256 — band width 160 >= 128+32 covers every
unordered pair from one side or the other):
  counts matmul (fp8, exact for 0/1 one-hots): C^T[jb, ia] = M_win^T M_blk
  ACT:  G = ln(C/512 + 1e-9)
  DVE:  F = (G + SHIFT) * C   (fp16, recentred so fp16 error is tiny;
        the eps*G term is dropped -- ~1e-5 relative, validated)
  PE:   indicator matmul contracts jb partitions -> R[j, ia] = sum_b F
  DVE:  grouped reduce over a -> S_raw^T[j_window, i]
  plus a tiny marginal path for pssm/conservation/t.
Host: gather, S = (S_raw - 512*SHIFT)/(512*ln2), mirror the band across
the diagonal, MI/APC assembly (numpy vector math on gathered results).

The j axis is padded A=21 -> 24 letters so jb rows tile 128 exactly
(pad rows have C=0 and F=0 so they are self-masking).
"""

import numpy as np
import ml_dtypes

import concourse.bass as bass
import concourse.mybir as mybir
import concourse.tile as tile
from concourse import bacc, bass_utils

F32 = mybir.dt.float32
F16 = mybir.dt.float16
FP8 = mybir.dt.float8e4  # e4m3: 0.0/1.0 exact
NP_FP8 = ml_dtypes.float8_e4m3

K, L, A = 512, 256, 21
AP24 = 24                  # padded alphabet for the jb (partition) axis
N_CORES = 8
IB = L // N_CORES          # 32 i-positions per core
NW = IB * A                # 672 rhs columns per core
JW = 160                   # j-window positions per core (wrap-around band)
MP = JW * AP24             # 3840 lhsT columns (jb, padded)
NPT = MP // 128            # 30 jb partition tiles
KT = K // 128              # 4 contraction tiles
NCH = 5                    # lhsT DMA chunks
CPT = NPT // NCH           # 6 ptiles per chunk
EPS = 1e-9
SHIFT = 6.0                # F = (ln J + SHIFT) * C recentring
LN2 = float(np.log(2.0))

_CACHE = {}


def _build():
    nc = bacc.Bacc("TRN2", target_bir_lowering=False, debug=False,
                   num_devices=N_CORES)

    # host-pretiled layouts: partition dim first, big contiguous runs
    lhst_d = nc.dram_tensor("lhst", [128, NCH, KT, CPT * 128], FP8,
                            kind="ExternalInput").ap()
    rhs_d = nc.dram_tensor("rhs", [128, KT, NW], FP8, kind="ExternalInput").ap()
    ind_d = nc.dram_tensor("ind", [128, 24, 128], F16, kind="ExternalInput").ap()
    sout_d = nc.dram_tensor("sout", [JW, IB], F32, kind="ExternalOutput").ap()
    pssm_d = nc.dram_tensor("pssm_raw", [1, NW], F32, kind="ExternalOutput").ap()
    traw_d = nc.dram_tensor("traw", [1, IB], F32, kind="ExternalOutput").ap()
    eraw_d = nc.dram_tensor("eraw", [1, IB], F32, kind="ExternalOutput").ap()

    with tile.TileContext(nc) as tc:
        with (
            tc.tile_pool(name="inp", bufs=1) as inp,
            tc.tile_pool(name="cpool", bufs=2, space="PSUM") as cpool,
            tc.tile_pool(name="rpool", bufs=1, space="PSUM") as rpool,
            tc.tile_pool(name="gpool", bufs=2) as gpool,
            tc.tile_pool(name="fpool", bufs=2) as fpool,
            tc.tile_pool(name="opool", bufs=1) as opool,
        ):
            rbuf = inp.tile([128, KT, NW], FP8)
            nc.sync.dma_start(rbuf[:], rhs_d[:])
            kchunks = []
            for c in range(NCH):
                kc = inp.tile([128, KT, CPT * 128], FP8, name=f"kchunk{c}")
                nc.sync.dma_start(kc[:], lhst_d[:, c, :, :])
                kchunks.append(kc)
            ibuf = inp.tile([128, 24, 128], F16)
            nc.sync.dma_start(ibuf[:], ind_d[:])

            bias1 = inp.tile([128, 1], F32)
            nc.vector.memset(bias1[:], EPS)
            biasp = inp.tile([128, 1], F32)
            nc.vector.memset(biasp[:], 0.001 * A / (1.0 + A * 0.001))
            ones8 = inp.tile([128, 1], FP8)
            nc.vector.memset(ones8[:], 1.0)

            r1a = rpool.tile([128, 1024], F32, tag="r1a")
            r1b = rpool.tile([128, 1024], F32, tag="r1b")

            for r in range(NPT):
                kc = kchunks[r // CPT]
                co = (r % CPT) * 128
                ctile = cpool.tile([128, 1024], F32, tag="c", name=f"c{r}")
                for k in range(KT):
                    lw = kc[:, k, co:co + 128]
                    nc.tensor.matmul(ctile[:, 0:512], lw, rbuf[:, k, 0:512],
                                     start=(k == 0), stop=(k == KT - 1))
                    nc.tensor.matmul(ctile[:, 512:NW], lw, rbuf[:, k, 512:NW],
                                     start=(k == 0), stop=(k == KT - 1))
                g = gpool.tile([128, NW], F32, tag="g", name=f"g{r}")
                nc.scalar.activation(g[:], ctile[:, 0:NW],
                                     mybir.ActivationFunctionType.Ln,
                                     scale=1.0 / K, bias=bias1[:])
                f = fpool.tile([128, NW], F16, tag="f", name=f"f{r}")
                nc.vector.scalar_tensor_tensor(
                    f[:], g[:], SHIFT, ctile[:, 0:NW],
                    op0=mybir.AluOpType.add, op1=mybir.AluOpType.mult)
                # indicator matmul: contract jb partitions -> per-j sums
                # ptiles 0..23 -> j 0..127 (r1a), ptiles 24..29 -> j 128..159 (r1b)
                rt = r1a if r < 24 else r1b
                ind_ap = ibuf[:, r % 24, :]
                nc.tensor.matmul(rt[:, 0:512], ind_ap, f[:, 0:512],
                                 start=(r % 24 == 0), stop=(r % 24 == 23 or r == NPT - 1))
                nc.tensor.matmul(rt[:, 512:NW], ind_ap, f[:, 512:NW],
                                 start=(r % 24 == 0), stop=(r % 24 == 23 or r == NPT - 1))

            sa = opool.tile([128, IB], F32)
            nc.vector.reduce_sum(sa[:], r1a[:, 0:NW].rearrange("p (i a) -> p i a", a=A),
                                 axis=mybir.AxisListType.X)
            sb = opool.tile([32, IB], F32)
            nc.vector.reduce_sum(sb[:], r1b[0:32, 0:NW].rearrange("p (i a) -> p i a", a=A),
                                 axis=mybir.AxisListType.X)
            nc.sync.dma_start(sout_d[0:128, :], sa[:])
            nc.sync.dma_start(sout_d[128:JW, :], sb[:])

            # marginal path: counts of the core's own columns
            cm = cpool.tile([1, 1024], F32, tag="c", name="cm")
            for k in range(KT):
                nc.tensor.matmul(cm[:, 0:512], ones8[:, 0:1], rbuf[:, k, 0:512],
                                 start=(k == 0), stop=(k == KT - 1))
                nc.tensor.matmul(cm[:, 512:NW], ones8[:, 0:1], rbuf[:, k, 512:NW],
                                 start=(k == 0), stop=(k == KT - 1))
            lnfe = gpool.tile([1, NW], F32, tag="g", name="lnfe")
            nc.scalar.activation(lnfe[:], cm[:, 0:NW],
                                 mybir.ActivationFunctionType.Ln,
                                 scale=1.0 / K, bias=bias1[0:1])
            pssm_t = fpool.tile([1, NW], F32, tag="f", name="pssm_t")
            nc.scalar.activation(pssm_t[:], cm[:, 0:NW],
                                 mybir.ActivationFunctionType.Ln,
                                 scale=A / (K * (1.0 + A * 0.001)),
                                 bias=biasp[0:1])
            nc.sync.dma_start(pssm_d[:, :], pssm_t[:])
            tv = fpool.tile([1, NW], F32, tag="f", name="tv")
            nc.vector.scalar_tensor_tensor(
                tv[:], cm[:, 0:NW], float(K * A * EPS), lnfe[:],
                op0=mybir.AluOpType.add, op1=mybir.AluOpType.mult)
            ev = gpool.tile([1, NW], F32, tag="g", name="ev")
            nc.vector.scalar_tensor_tensor(
                ev[:], cm[:, 0:NW], float(K * EPS), lnfe[:],
                op0=mybir.AluOpType.add, op1=mybir.AluOpType.mult)
            tr = opool.tile([1, IB], F32)
            nc.vector.reduce_sum(tr[:], tv[:].rearrange("p (i a) -> p i a", a=A),
                                 axis=mybir.AxisListType.X)
            er = opool.tile([1, IB], F32)
            nc.vector.reduce_sum(er[:], ev[:].rearrange("p (i a) -> p i a", a=A),
                                 axis=mybir.AxisListType.X)
            nc.sync.dma_start(traw_d[:, :], tr[:])
            nc.sync.dma_start(eraw_d[:, :], er[:])

    nc.compile()
    return nc


def _indicator():
    # ind[p, r, j] = 1 iff (128*r + p) // 24 == j (mod 128 within group)
    ind = np.zeros((128, 24, 128), np.float16)
    r = np.arange(24)[None, :]
    p = np.arange(128)[:, None]
    j = (128 * r + p) // AP24
    ind[p, r, j] = 1.0
    return ind


def run(msa: np.ndarray, trace: bool = False):
    """Shard, run the SPMD kernel on 8 cores, return BassKernelResults."""
    if "nc" not in _CACHE:
        _CACHE["nc"] = _build()
        _CACHE["ind"] = _indicator()
    nc = _CACHE["nc"]

    msa = np.asarray(msa, dtype=np.float32)
    m21 = msa.reshape(K, L * A).astype(NP_FP8)
    m24 = np.zeros((K, L, AP24), NP_FP8)
    m24[:, :, :A] = msa
    ind = _CACHE["ind"]

    in_maps = []
    for c in range(N_CORES):
        jpos = (32 * c + np.arange(JW)) % L
        win = m24[:, jpos, :].reshape(K, MP)          # [512, 3840]
        # pretile: lhst[p, chunk, k, x] = win[128k + p, 768*chunk + x]
        lh = np.ascontiguousarray(
            win.reshape(KT, 128, NCH, CPT * 128).transpose(1, 2, 0, 3))
        rh = np.ascontiguousarray(
            m21[:, c * NW:(c + 1) * NW].reshape(KT, 128, NW).transpose(1, 0, 2))
        in_maps.append({"lhst": lh, "rhs": rh, "ind": ind})

    return bass_utils.run_bass_kernel_spmd(
        nc, in_maps, core_ids=list(range(N_CORES)), trace=trace)


def assemble(res) -> tuple[np.ndarray, np.ndarray, np.ndarray]:
    inv = 1.0 / (K * LN2)
    # core c: sout[jw, i_local] -> S[32c + i_local, (32c + jw) % L]
    s = np.zeros((L, L), np.float32)
    covered = np.zeros((L, L), bool)
    jw = np.arange(JW)
    for c in range(N_CORES):
        jpos = (32 * c + jw) % L
        blk = res.results[c]["sout"]                  # [JW, IB]
        s[32 * c:32 * c + IB, jpos] = blk.T
        covered[32 * c:32 * c + IB, jpos] = True
    s = np.where(covered, s, s.T)
    s = (s - np.float32(K * SHIFT)) * np.float32(inv)

    t = np.concatenate([res.results[c]["traw"][0] for c in range(N_CORES)]) * np.float32(inv)
    e = np.concatenate([res.results[c]["eraw"][0] for c in range(N_CORES)])
    pssm = np.concatenate([res.results[c]["pssm_raw"][0] for c in range(N_CORES)])

    mi = s - t[:, None] - t[None, :]
    np.fill_diagonal(mi, 0.0)
    rm = mi.mean(axis=1)
    cm = mi.mean(axis=0)
    tm = mi.mean()
    mi = mi - np.outer(rm, cm) / (tm + EPS)

    pssm = (pssm / np.float32(LN2)).reshape(L, A).astype(np.float32)
    conservation = (1.0 + e * inv / np.log2(A)).astype(np.float32)
    return pssm, conservation, mi.astype(np.float32)


def kernel(msa: np.ndarray):
    res = run(np.asarray(msa, dtype=np.float32), trace=False)
    return assemble(res)


# revision 5
# speedup vs baseline: 1.7445x; 1.1337x over previous
"""Trainium2 Bass kernel for nn_EvolutionaryFeatureExtractor.

Reference computes, from a one-hot MSA (K=512, L=256, A=21):
  pssm         (L, A)  = log2(((mean + 0.001)/rowsum) * A)
  conservation (L,)    = 1 - entropy/log2(A)
  coevolution  (L, L)  = APC-corrected mutual information

Algebraic restructure (validated against the jax reference):
  joint[i,j,a,b] = C[i,j,a,b]/K + EPS with C the integer pair counts.
  sum_b joint[i,j,a,b] = p_raw[i,a] + A*EPS  (independent of j), so the
  two einsum('ijab,ia->ij') terms collapse to rank-1: MI = S - t_i - t_j
  with S[i,j] = sum_ab J*log2(J) and t[i] = sum_a (p_raw+A*EPS)*log2(p_raw+EPS).
  S is symmetric, so each core only computes a wrap-around band of j.

Device pipeline per core (core c owns i-positions [32c, 32c+32) and the
j-window [32c, 32c+160) mod 256 — band width 160 >= 128+32 covers every
unordered pair from one side or the other):
  counts matmul (fp8, exact for 0/1 one-hots): C^T[jb, ia] = M_win^T M_blk
  ACT:  G = ln(C/512 + 1e-9)
  DVE:  F = (G + SHIFT) * C   (fp16, recentred so fp16 error is tiny;
        the eps*G term is dropped -- ~1e-5 relative, validated)
  PE:   indicator matmul contracts jb partitions -> R[j, ia] = sum_b F
  DVE:  grouped reduce over a -> S_raw^T[j_window, i]
  plus a tiny marginal path for pssm/conservation/t.
Host: gather, S = (S_raw - 512*SHIFT)/(512*ln2), mirror the band across
the diagonal, MI/APC assembly (numpy vector math on gathered results).

The j axis is padded A=21 -> 24 letters so jb rows tile 128 exactly
(pad rows have C=0 and F=0 so they are self-masking).
"""

import numpy as np
import ml_dtypes

import concourse.bass as bass
import concourse.mybir as mybir
import concourse.tile as tile
from concourse import bacc, bass_utils

F32 = mybir.dt.float32
F16 = mybir.dt.float16
FP8 = mybir.dt.float8e4  # e4m3: 0.0/1.0 exact
NP_FP8 = ml_dtypes.float8_e4m3

K, L, A = 512, 256, 21
AP24 = 24                  # padded alphabet for the jb (partition) axis
N_CORES = 8
IB = L // N_CORES          # 32 i-positions per core
NW = IB * A                # 672 rhs columns per core
JW = 160                   # j-window positions per core (wrap-around band)
MP = JW * AP24             # 3840 lhsT columns (jb, padded)
NPT = MP // 128            # 30 jb partition tiles
KT = K // 128              # 4 contraction tiles
NCH = 5                    # lhsT DMA chunks
CPT = NPT // NCH           # 6 ptiles per chunk
EPS = 1e-9
SHIFT = 6.0                # F = (ln J + SHIFT) * C recentring
LN2 = float(np.log(2.0))

_CACHE = {}


def _build():
    nc = bacc.Bacc("TRN2", target_bir_lowering=False, debug=False,
                   num_devices=N_CORES)

    # host-pretiled layouts: partition dim first, big contiguous runs
    lhst_d = nc.dram_tensor("lhst", [128, NCH, KT, CPT * 128], FP8,
                            kind="ExternalInput").ap()
    rhs_d = nc.dram_tensor("rhs", [128, KT, NW], FP8, kind="ExternalInput").ap()
    ind_d = nc.dram_tensor("ind", [128, 24, 128], F16, kind="ExternalInput").ap()
    sout_d = nc.dram_tensor("sout", [JW, IB], F32, kind="ExternalOutput").ap()
    pssm_d = nc.dram_tensor("pssm_raw", [1, NW], F32, kind="ExternalOutput").ap()
    traw_d = nc.dram_tensor("traw", [1, IB], F32, kind="ExternalOutput").ap()
    eraw_d = nc.dram_tensor("eraw", [1, IB], F32, kind="ExternalOutput").ap()

    with tile.TileContext(nc) as tc:
        with (
            tc.tile_pool(name="inp", bufs=1) as inp,
            tc.tile_pool(name="cpool", bufs=2, space="PSUM") as cpool,
            tc.tile_pool(name="rpool", bufs=1, space="PSUM") as rpool,
            tc.tile_pool(name="gpool", bufs=2) as gpool,
            tc.tile_pool(name="fpool", bufs=4) as fpool,
            tc.tile_pool(name="opool", bufs=1) as opool,
        ):
            # input DMAs spread across engine DGE queues so they overlap
            rbuf = inp.tile([128, KT, NW], FP8)
            nc.sync.dma_start(rbuf[:], rhs_d[:])
            kchunks = []
            dma_eng = [nc.scalar, nc.gpsimd, nc.sync, nc.scalar, nc.gpsimd]
            for c in range(NCH):
                kc = inp.tile([128, KT, CPT * 128], FP8, name=f"kchunk{c}")
                dma_eng[c].dma_start(kc[:], lhst_d[:, c, :, :])
                kchunks.append(kc)
            ibuf = inp.tile([128, 24, 128], F16)
            nc.scalar.dma_start(ibuf[:], ind_d[:])

            bias1 = inp.tile([128, 1], F32)
            nc.vector.memset(bias1[:], EPS)
            biasp = inp.tile([128, 1], F32)
            nc.vector.memset(biasp[:], 0.001 * A / (1.0 + A * 0.001))
            ones8 = inp.tile([128, 1], FP8)
            nc.vector.memset(ones8[:], 1.0)

            r1a = rpool.tile([128, 1024], F32, tag="r1a")
            r1b = rpool.tile([128, 1024], F32, tag="r1b")

            # marginal path first: needs only rbuf, warms the PE while the
            # big lhsT chunks are still in flight
            cm = cpool.tile([1, 1024], F32, tag="c", name="cm")
            for k in range(KT):
                nc.tensor.matmul(cm[:, 0:512], ones8[:, 0:1], rbuf[:, k, 0:512],
                                 start=(k == 0), stop=(k == KT - 1))
                nc.tensor.matmul(cm[:, 512:NW], ones8[:, 0:1], rbuf[:, k, 512:NW],
                                 start=(k == 0), stop=(k == KT - 1))
            lnfe = gpool.tile([1, NW], F32, tag="g", name="lnfe")
            nc.scalar.activation(lnfe[:], cm[:, 0:NW],
                                 mybir.ActivationFunctionType.Ln,
                                 scale=1.0 / K, bias=bias1[0:1])
            pssm_t = fpool.tile([1, NW], F32, tag="f", name="pssm_t")
            nc.scalar.activation(pssm_t[:], cm[:, 0:NW],
                                 mybir.ActivationFunctionType.Ln,
                                 scale=A / (K * (1.0 + A * 0.001)),
                                 bias=biasp[0:1])
            nc.sync.dma_start(pssm_d[:, :], pssm_t[:])
            tv = fpool.tile([1, NW], F32, tag="f", name="tv")
            nc.vector.scalar_tensor_tensor(
                tv[:], cm[:, 0:NW], float(K * A * EPS), lnfe[:],
                op0=mybir.AluOpType.add, op1=mybir.AluOpType.mult)
            ev = gpool.tile([1, NW], F32, tag="g", name="ev")
            nc.vector.scalar_tensor_tensor(
                ev[:], cm[:, 0:NW], float(K * EPS), lnfe[:],
                op0=mybir.AluOpType.add, op1=mybir.AluOpType.mult)
            tr = opool.tile([1, IB], F32)
            nc.vector.reduce_sum(tr[:], tv[:].rearrange("p (i a) -> p i a", a=A),
                                 axis=mybir.AxisListType.X)
            er = opool.tile([1, IB], F32)
            nc.vector.reduce_sum(er[:], ev[:].rearrange("p (i a) -> p i a", a=A),
                                 axis=mybir.AxisListType.X)
            nc.sync.dma_start(traw_d[:, :], tr[:])
            nc.sync.dma_start(eraw_d[:, :], er[:])

            # main loop; the indicator matmul for ptile r is emitted DELAY
            # iterations later so it never head-of-line-blocks the PE queue
            # waiting on the DVE to produce F
            DELAY = 2
            fs = [None] * NPT

            def emit_m1(r):
                rt = r1a if r < 24 else r1b
                ind_ap = ibuf[:, r % 24, :]
                nc.tensor.matmul(rt[:, 0:512], ind_ap, fs[r][:, 0:512],
                                 start=(r % 24 == 0),
                                 stop=(r % 24 == 23 or r == NPT - 1))
                nc.tensor.matmul(rt[:, 512:NW], ind_ap, fs[r][:, 512:NW],
                                 start=(r % 24 == 0),
                                 stop=(r % 24 == 23 or r == NPT - 1))

            for r in range(NPT):
                kc = kchunks[r // CPT]
                co = (r % CPT) * 128
                ctile = cpool.tile([128, 1024], F32, tag="c", name=f"c{r}")
                for k in range(KT):
                    lw = kc[:, k, co:co + 128]
                    nc.tensor.matmul(ctile[:, 0:512], lw, rbuf[:, k, 0:512],
                                     start=(k == 0), stop=(k == KT - 1))
                    nc.tensor.matmul(ctile[:, 512:NW], lw, rbuf[:, k, 512:NW],
                                     start=(k == 0), stop=(k == KT - 1))
                if r >= DELAY:
                    emit_m1(r - DELAY)
                g = gpool.tile([128, NW], F32, tag="g", name=f"g{r}")
                nc.scalar.activation(g[:], ctile[:, 0:NW],
                                     mybir.ActivationFunctionType.Ln,
                                     scale=1.0 / K, bias=bias1[:])
                f = fpool.tile([128, NW], F16, tag="f", name=f"f{r}")
                fs[r] = f
                nc.vector.scalar_tensor_tensor(
                    f[:], g[:], SHIFT, ctile[:, 0:NW],
                    op0=mybir.AluOpType.add, op1=mybir.AluOpType.mult)
            for r in range(NPT - DELAY, NPT):
                emit_m1(r)

            sa = opool.tile([128, IB], F32)
            nc.vector.reduce_sum(sa[:], r1a[:, 0:NW].rearrange("p (i a) -> p i a", a=A),
                                 axis=mybir.AxisListType.X)
            sb = opool.tile([32, IB], F32)
            nc.vector.reduce_sum(sb[:], r1b[0:32, 0:NW].rearrange("p (i a) -> p i a", a=A),
                                 axis=mybir.AxisListType.X)
            nc.sync.dma_start(sout_d[0:128, :], sa[:])
            nc.sync.dma_start(sout_d[128:JW, :], sb[:])

    nc.compile()
    return nc


def _indicator():
    # ind[p, r, j] = 1 iff (128*r + p) // 24 == j (mod 128 within group)
    ind = np.zeros((128, 24, 128), np.float16)
    r = np.arange(24)[None, :]
    p = np.arange(128)[:, None]
    j = (128 * r + p) // AP24
    ind[p, r, j] = 1.0
    return ind


def run(msa: np.ndarray, trace: bool = False):
    """Shard, run the SPMD kernel on 8 cores, return BassKernelResults."""
    if "nc" not in _CACHE:
        _CACHE["nc"] = _build()
        _CACHE["ind"] = _indicator()
    nc = _CACHE["nc"]

    msa = np.asarray(msa, dtype=np.float32)
    m21 = msa.reshape(K, L * A).astype(NP_FP8)
    m24 = np.zeros((K, L, AP24), NP_FP8)
    m24[:, :, :A] = msa
    ind = _CACHE["ind"]

    in_maps = []
    for c in range(N_CORES):
        jpos = (32 * c + np.arange(JW)) % L
        win = m24[:, jpos, :].reshape(K, MP)          # [512, 3840]
        # pretile: lhst[p, chunk, k, x] = win[128k + p, 768*chunk + x]
        lh = np.ascontiguousarray(
            win.reshape(KT, 128, NCH, CPT * 128).transpose(1, 2, 0, 3))
        rh = np.ascontiguousarray(
            m21[:, c * NW:(c + 1) * NW].reshape(KT, 128, NW).transpose(1, 0, 2))
        in_maps.append({"lhst": lh, "rhs": rh, "ind": ind})

    return bass_utils.run_bass_kernel_spmd(
        nc, in_maps, core_ids=list(range(N_CORES)), trace=trace)


def assemble(res) -> tuple[np.ndarray, np.ndarray, np.ndarray]:
    inv = 1.0 / (K * LN2)
    # core c: sout[jw, i_local] -> S[32c + i_local, (32c + jw) % L]
    s = np.zeros((L, L), np.float32)
    covered = np.zeros((L, L), bool)
    jw = np.arange(JW)
    for c in range(N_CORES):
        jpos = (32 * c + jw) % L
        blk = res.results[c]["sout"]                  # [JW, IB]
        s[32 * c:32 * c + IB, jpos] = blk.T
        covered[32 * c:32 * c + IB, jpos] = True
    s = np.where(covered, s, s.T)
    s = (s - np.float32(K * SHIFT)) * np.float32(inv)

    t = np.concatenate([res.results[c]["traw"][0] for c in range(N_CORES)]) * np.float32(inv)
    e = np.concatenate([res.results[c]["eraw"][0] for c in range(N_CORES)])
    pssm = np.concatenate([res.results[c]["pssm_raw"][0] for c in range(N_CORES)])

    mi = s - t[:, None] - t[None, :]
    np.fill_diagonal(mi, 0.0)
    rm = mi.mean(axis=1)
    cm = mi.mean(axis=0)
    tm = mi.mean()
    mi = mi - np.outer(rm, cm) / (tm + EPS)

    pssm = (pssm / np.float32(LN2)).reshape(L, A).astype(np.float32)
    conservation = (1.0 + e * inv / np.log2(A)).astype(np.float32)
    return pssm, conservation, mi.astype(np.float32)


def kernel(msa: np.ndarray):
    res = run(np.asarray(msa, dtype=np.float32), trace=False)
    return assemble(res)
